# revision 12
# baseline (speedup 1.0000x reference)
"""Trainium2 Bass kernel for nn_MoE_61993557950953.

MoE with dense routing math: B=16384 tokens, D=512, E=16 experts, H=128,
O=512, top-2 gating + aux load-balancing loss.

Strategy: data-parallel over tokens across 8 NeuronCores (2048 tokens/core),
expert weights replicated. Host pre-transposes x so the kernel needs no
input transposes. Per 128-token tile: gate matmul + dense all-expert mm1
(f32r), top-2 via DVE max8 + equality masks, exact-gelu LUT with fused
row-sum (LN mean), Square+accum (LN var), LN scale and gate weight folded
into one scale/bias applied to h, per-expert PE transpose, mm2 accumulated
over all 16 experts in PSUM. Aux-loss partial sums are computed on-chip and
finalized on host.
"""
import sys

sys.path.insert(0, "/opt/trn_rl_repo")

import numpy as np

import concourse.bass as bass
import concourse.mybir as mybir
import concourse.tile as tile
from concourse import bacc
from concourse.alu_op_type import AluOpType
from concourse.masks import make_identity

AF = mybir.ActivationFunctionType
dt = mybir.dt

# Swappable for CoreSim testing (sim has no Gelu LUT)
GELU_FUNC = AF.Gelu

# Problem shape (hardcoded; harness runs this exact problem)
B, D, E, H, O = 16384, 512, 16, 128, 512
NCORES = 8
BC = B // NCORES          # tokens per core
P = 128                   # partitions / token tile
NT = BC // P              # token tiles per core
KC = D // P               # contraction chunks (4)
LN_EPS = 1e-5

MM = dt.float32r          # matmul dtype (f32r: full PE speed, ~1e-4 rel err)
F32 = dt.float32


def build_program(has_b1: bool, has_gb: bool, has_b2: bool):
    nc = bacc.Bacc(None, target_bir_lowering=False)

    xt = nc.dram_tensor("xt", [KC, P, BC], F32, kind="ExternalInput")
    w1 = nc.dram_tensor("w1", [KC, P, E * H], MM, kind="ExternalInput")
    w2 = nc.dram_tensor("w2", [H, E * O], MM, kind="ExternalInput")
    gw = nc.dram_tensor("gw", [KC, P, E], F32, kind="ExternalInput")
    if has_b1:
        b1d = nc.dram_tensor("b1d", [1, E * H], MM, kind="ExternalInput")
    if has_gb:
        gbd = nc.dram_tensor("gbd", [1, E], F32, kind="ExternalInput")
    if has_b2:
        b2d = nc.dram_tensor("b2d", [E, O], MM, kind="ExternalInput")

    out = nc.dram_tensor("out", [BC, O], F32, kind="ExternalOutput")
    stats = nc.dram_tensor("stats", [P, 17], F32, kind="ExternalOutput")

    with tile.TileContext(nc) as tc:
        with (
            tc.tile_pool(name="wpool", bufs=1) as wpool,
            tc.tile_pool(name="xpool", bufs=3) as xpool,
            tc.tile_pool(name="hpool", bufs=2) as hpool,
            tc.tile_pool(name="spool", bufs=2) as spool,
            tc.tile_pool(name="upool", bufs=3) as upool,
            tc.tile_pool(name="opool", bufs=2) as opool,
            tc.tile_pool(name="ph", bufs=1, space=bass.MemorySpace.PSUM) as php,
            tc.tile_pool(name="pout", bufs=1, space=bass.MemorySpace.PSUM) as poutp,
            tc.tile_pool(name="psc", bufs=2, space=bass.MemorySpace.PSUM) as pscp,
            tc.tile_pool(name="ptr", bufs=1, space=bass.MemorySpace.PSUM) as ptrp,
        ):
            # ---------------- persistent weights ----------------
            w1_s = wpool.tile([P, KC, E * H], MM)
            for k in range(KC):
                nc.sync.dma_start(w1_s[:, k, :], w1[k])
            w2_s = wpool.tile([P, E * O], MM)
            nc.sync.dma_start(w2_s[:], w2[:])
            gw_s = wpool.tile([P, KC, E], F32)
            for k in range(KC):
                nc.sync.dma_start(gw_s[:, k, :], gw[k])
            ident_f = wpool.tile([P, P], F32)
            make_identity(nc, ident_f[:])
            ident_r = wpool.tile([P, P], MM)
            nc.vector.tensor_copy(ident_r[:], ident_f[:])
            ident = ident_r[:]
            if has_b1:
                ones_f = wpool.tile([1, P], F32)
                nc.vector.memset(ones_f[:], 1.0)
                ones_t = wpool.tile([1, P], MM)
                nc.vector.tensor_copy(ones_t[:], ones_f[:])
                ones_r = ones_t[:]
                b1_s = wpool.tile([1, E * H], MM)
                nc.sync.dma_start(b1_s[:], b1d[:])
            if has_gb:
                if not has_b1:
                    ones_f = wpool.tile([1, P], F32)
                    nc.vector.memset(ones_f[:], 1.0)
                    ones_t = wpool.tile([1, P], MM)
                    nc.vector.tensor_copy(ones_t[:], ones_f[:])
                    ones_r = ones_t[:]
                gb_s = wpool.tile([1, E], F32)
                nc.sync.dma_start(gb_s[:], gbd[:])
            if has_b2:
                b2_s = wpool.tile([E, O], MM)
                nc.sync.dma_start(b2_s[:], b2d[:])

            # aux-loss accumulators
            macc = wpool.tile([P, E], F32)
            nc.vector.memset(macc[:], 0.0)
            eacc = wpool.tile([P, 1], F32)
            nc.vector.memset(eacc[:], 0.0)

            NEG = -1e30

            for t in range(NT):
                tsl = bass.ts(t, P)
                # ---------------- load x^T tile ----------------
                xt_t = xpool.tile([P, KC, P], F32)
                for k in range(KC):
                    nc.sync.dma_start(xt_t[:, k, :], xt[k, :, tsl])

                # ---------------- gate + mm1 ----------------
                ps_s = pscp.tile([P, E], F32)
                ps_h = php.tile([P, E * H], F32)
                xr_t = xpool.tile([P, KC, P], MM, tag="xr")
                nc.vector.tensor_copy(xr_t[:], xt_t[:])
                for k in range(KC):
                    lhs = xr_t[:, k, :]
                    nc.tensor.matmul(ps_s[:], xt_t[:, k, :], gw_s[:, k, :],
                                     start=(k == 0), stop=(k == KC - 1 and not has_gb))
                    for g in range(4):
                        nc.tensor.matmul(ps_h[:, bass.ts(g, 512)], lhs,
                                         w1_s[:, k, bass.ts(g, 512)],
                                         start=(k == 0),
                                         stop=(k == KC - 1 and not has_b1))
                if has_gb:
                    nc.tensor.matmul(ps_s[:], ones_f[:], gb_s[:], start=False, stop=True)
                if has_b1:
                    for g in range(4):
                        nc.tensor.matmul(ps_h[:, bass.ts(g, 512)], ones_r,
                                         b1_s[:, bass.ts(g, 512)], start=False, stop=True)

                # ---------------- routing ----------------
                s_t = spool.tile([P, E], F32)
                nc.vector.tensor_copy(s_t[:], ps_s[:])
                t8 = spool.tile([P, 8], F32)
                nc.vector.max(out=t8[:], in_=s_t[:])
                v0 = t8[:, 0:1]
                v1 = t8[:, 1:2]
                # top-2 softmax weights
                dfe = spool.tile([P, 1], F32)
                nc.vector.tensor_tensor(dfe[:], v1, v0, op=AluOpType.subtract)
                ew = spool.tile([P, 1], F32)
                nc.scalar.activation(ew[:], dfe[:], AF.Exp)
                zs = spool.tile([P, 1], F32)
                nc.vector.tensor_scalar_add(zs[:], ew[:], 1.0)
                rz = spool.tile([P, 1], F32)
                nc.vector.reciprocal(rz[:], zs[:])
                w1g = spool.tile([P, 1], F32)
                nc.vector.tensor_tensor(w1g[:], ew[:], rz[:], op=AluOpType.mult)
                w0g = spool.tile([P, 1], F32)
                nc.vector.tensor_scalar(w0g[:], w1g[:], -1.0, 1.0,
                                        op0=AluOpType.mult, op1=AluOpType.add)
                m0 = spool.tile([P, E], F32)
                m1 = spool.tile([P, E], F32)
                nc.vector.tensor_scalar(m0[:], s_t[:], v0, None, op0=AluOpType.is_equal)
                nc.vector.tensor_scalar(m1[:], s_t[:], v1, None, op0=AluOpType.is_equal)
                wg_t = spool.tile([P, E], F32)
                tt0 = spool.tile([P, E], F32)
                nc.vector.tensor_scalar(tt0[:], m0[:], w0g[:], None, op0=AluOpType.mult)
                nc.vector.tensor_scalar(wg_t[:], m1[:], w1g[:], None, op0=AluOpType.mult)
                nc.vector.tensor_add(wg_t[:], wg_t[:], tt0[:])
                # usage accumulation
                nc.vector.tensor_add(macc[:], macc[:], m0[:])
                nc.vector.tensor_add(macc[:], macc[:], m1[:])
                # entropy of full softmax
                nv0 = spool.tile([P, 1], F32)
                nc.vector.tensor_scalar(nv0[:], v0, -1.0, None, op0=AluOpType.mult)
                e1 = spool.tile([P, E], F32)
                zf = spool.tile([P, 1], F32)
                nc.scalar.activation(e1[:], s_t[:], AF.Exp, bias=nv0[:], accum_out=zf[:])
                smul = spool.tile([P, E], F32)
                nc.vector.tensor_tensor(smul[:], s_t[:], e1[:], op=AluOpType.mult)
                tsum = spool.tile([P, 1], F32)
                nc.vector.reduce_sum(tsum[:], smul[:], axis=mybir.AxisListType.X)
                lz = spool.tile([P, 1], F32)
                nc.scalar.activation(lz[:], zf[:], AF.Ln)
                rzf = spool.tile([P, 1], F32)
                nc.vector.reciprocal(rzf[:], zf[:])
                qq = spool.tile([P, 1], F32)
                nc.vector.tensor_tensor(qq[:], tsum[:], rzf[:], op=AluOpType.mult)
                ent = spool.tile([P, 1], F32)
                nc.vector.tensor_tensor(ent[:], v0, lz[:], op=AluOpType.add)
                nc.vector.tensor_tensor(ent[:], ent[:], qq[:], op=AluOpType.subtract)
                nc.vector.tensor_add(eacc[:], eacc[:], ent[:])

                # ---------------- gelu + LN stats ----------------
                hg = hpool.tile([P, E, H], MM)
                S = spool.tile([P, E], F32)
                SS = spool.tile([P, E], F32)
                sqscr = hpool.tile([P, H], MM, tag="sqscr")
                for e in range(E):
                    nc.scalar.activation(hg[:, e, :], ps_h[:, bass.ts(e, H)], GELU_FUNC,
                                         accum_out=S[:, e:e + 1])
                for e in range(E):
                    nc.scalar.activation(sqscr[:], hg[:, e, :], AF.Square,
                                         accum_out=SS[:, e:e + 1])

                # batched LN math: A = wg * rsqrt(var+eps), Cn = mu * A
                mu = spool.tile([P, E], F32)
                nc.vector.tensor_scalar(mu[:], S[:], 1.0 / H, None, op0=AluOpType.mult)
                varpe = spool.tile([P, E], F32)
                nc.vector.tensor_scalar(varpe[:], SS[:], 1.0 / H, LN_EPS,
                                        op0=AluOpType.mult, op1=AluOpType.add)
                musq = spool.tile([P, E], F32)
                nc.vector.tensor_tensor(musq[:], mu[:], mu[:], op=AluOpType.mult)
                nc.vector.tensor_tensor(varpe[:], varpe[:], musq[:], op=AluOpType.subtract)
                rec = spool.tile([P, E], F32)
                nc.vector.reciprocal(rec[:], varpe[:])
                inv = spool.tile([P, E], F32)
                nc.scalar.activation(inv[:], rec[:], AF.Sqrt)
                A = spool.tile([P, E], F32)
                nc.vector.tensor_tensor(A[:], wg_t[:], inv[:], op=AluOpType.mult)
                Cn = spool.tile([P, E], F32)
                nc.vector.tensor_tensor(Cn[:], mu[:], A[:], op=AluOpType.mult)

                # ---------------- u = A*hg - Cn (broadcast along H) ----------------
                u_all = hpool.tile([P, E, H], MM)
                A3 = A[:, :, None].to_broadcast([P, E, H])
                C3 = Cn[:, :, None].to_broadcast([P, E, H])
                nc.vector.tensor_tensor(u_all[:], hg[:], A3, op=AluOpType.mult)
                nc.vector.tensor_tensor(u_all[:], u_all[:], C3, op=AluOpType.subtract)

                # ---------------- transpose + mm2 ----------------
                ps_o = poutp.tile([P, O], F32)
                for e in range(E):
                    ps_t = ptrp.tile([P, P], MM)
                    nc.tensor.transpose(ps_t[:], u_all[:, e, :], ident)
                    uT = upool.tile([P, P], MM)
                    if e % 2 == 0:
                        nc.vector.tensor_copy(uT[:], ps_t[:])
                    else:
                        nc.scalar.copy(uT[:], ps_t[:])
                    nc.tensor.matmul(ps_o[:], uT[:], w2_s[:, bass.ts(e, O)],
                                     start=(e == 0), stop=(e == E - 1 and not has_b2))
                if has_b2:
                    ps_wt = ptrp.tile([P, P], MM, tag="ps_wt")
                    wgr = spool.tile([P, E], MM, tag="wgr")
                    nc.vector.tensor_copy(wgr[:], wg_t[:])
                    nc.tensor.transpose(ps_wt[:E, :], wgr[:], ident)
                    wgT = upool.tile([E, P], MM, tag="wgT")
                    nc.vector.tensor_copy(wgT[:], ps_wt[:E, :])
                    nc.tensor.matmul(ps_o[:], wgT[:], b2_s[:], start=False, stop=True)

                o_t = opool.tile([P, O], F32)
                nc.vector.tensor_copy(o_t[:], ps_o[:])
                nc.sync.dma_start(out[tsl, :], o_t[:])

            # ---------------- stats out ----------------
            nc.sync.dma_start(stats[:, 0:E], macc[:])
            nc.sync.dma_start(stats[:, E:E + 1], eacc[:])

    nc.compile()
    return nc


_CACHE = {}


def _get_program(has_b1, has_gb, has_b2):
    key = (has_b1, has_gb, has_b2)
    if key not in _CACHE:
        _CACHE[key] = build_program(*key)
    return _CACHE[key]


def kernel(x, gate_W, gate_b, W1, b1, ln_g, ln_b, W2, b2):
    x = np.asarray(x, dtype=np.float32)
    gate_W = np.asarray(gate_W, dtype=np.float32)
    gate_b = np.asarray(gate_b, dtype=np.float32)
    W1 = np.asarray(W1, dtype=np.float32)
    b1 = np.asarray(b1, dtype=np.float32)
    ln_g = np.asarray(ln_g, dtype=np.float32)
    ln_b = np.asarray(ln_b, dtype=np.float32)
    W2 = np.asarray(W2, dtype=np.float32)
    b2 = np.asarray(b2, dtype=np.float32)

    has_b1 = bool(np.any(b1 != 0.0))
    has_gb = bool(np.any(gate_b != 0.0))
    # fold ln_g into W2; ln_b and b2 fold into a rank-1 output correction
    W2g = (ln_g[:, :, None] * W2).astype(np.float32)          # [E, H, O]
    B2 = (b2 + np.einsum("eh,eho->eo", ln_b, W2)).astype(np.float32)  # [E, O]
    has_b2 = bool(np.any(B2 != 0.0))

    nc = _get_program(has_b1, has_gb, has_b2)

    # host-side layout prep (weights shared by all cores)
    xT = np.ascontiguousarray(x.T)                             # [D, B]
    w1h = np.ascontiguousarray(
        W1.transpose(1, 0, 2).reshape(D, E * H).reshape(KC, P, E * H))
    w2h = np.ascontiguousarray(W2g.transpose(1, 0, 2).reshape(H, E * O))
    gwh = np.ascontiguousarray(gate_W.reshape(KC, P, E))

    in_maps = []
    for c in range(NCORES):
        m = {
            "xt": np.ascontiguousarray(
                xT[:, c * BC:(c + 1) * BC].reshape(KC, P, BC)),
            "w1": w1h,
            "w2": w2h,
            "gw": gwh,
        }
        if has_b1:
            m["b1d"] = np.ascontiguousarray(b1.reshape(1, E * H))
        if has_gb:
            m["gbd"] = np.ascontiguousarray(gate_b.reshape(1, E))
        if has_b2:
            m["b2d"] = B2
        in_maps.append(m)

    from concourse.bass_utils import run_bass_kernel_spmd
    res = run_bass_kernel_spmd(nc, in_maps, core_ids=list(range(NCORES)))

    out = np.concatenate([np.asarray(r["out"]) for r in res.results], axis=0)

    # aux loss from per-core stats
    usage_counts = np.zeros(E, dtype=np.float64)
    ent_sum = 0.0
    for r in res.results:
        st = np.asarray(r["stats"])
        usage_counts += st[:, 0:E].sum(axis=0, dtype=np.float64)
        ent_sum += st[:, E].sum(dtype=np.float64)
    expert_usage = (usage_counts / B).astype(np.float32)
    load_balance = np.mean((expert_usage - 1.0 / E) ** 2, dtype=np.float32)
    entropy = np.float32(ent_sum / B)
    aux_loss = np.float32(load_balance - 0.1 * entropy)

    return out, aux_loss


# revision 13
# speedup vs baseline: 244.0414x; 244.0414x over previous
"""Trainium2 Bass kernel for nn_MoE_61993557950953.

MoE with dense routing math: B=16384 tokens, D=512, E=16 experts, H=128,
O=512, top-2 gating + aux load-balancing loss.

Strategy: data-parallel over tokens across 8 NeuronCores (2048 tokens/core),
expert weights replicated. Host pre-transposes x so the kernel needs no
input transposes. Per 128-token tile: gate matmul + dense all-expert mm1
(f32r), top-2 via DVE max8 + equality masks, exact-gelu LUT with fused
row-sum (LN mean), Square+accum (LN var), LN scale and gate weight folded
into one scale/bias applied to h, per-expert PE transpose, mm2 accumulated
over all 16 experts in PSUM. Aux-loss partial sums are computed on-chip and
finalized on host.
"""
import sys

sys.path.insert(0, "/opt/trn_rl_repo")

import numpy as np

import concourse.bass as bass
import concourse.mybir as mybir
import concourse.tile as tile
from concourse import bacc
from concourse.alu_op_type import AluOpType
from concourse.masks import make_identity

AF = mybir.ActivationFunctionType
dt = mybir.dt

# Swappable for CoreSim testing (sim has no Gelu LUT)
GELU_FUNC = AF.Gelu
# In-NEFF repetition count (timing only; graded path uses 1)
REPEAT = 1

# Problem shape (hardcoded; harness runs this exact problem)
B, D, E, H, O = 16384, 512, 16, 128, 512
NCORES = 8
BC = B // NCORES          # tokens per core
P = 128                   # partitions / token tile
NT = BC // P              # token tiles per core
KC = D // P               # contraction chunks (4)
LN_EPS = 1e-5

MM = dt.float32r          # matmul dtype (f32r: full PE speed, ~1e-4 rel err)
F32 = dt.float32


def build_program(has_b1: bool, has_gb: bool, has_b2: bool):
    nc = bacc.Bacc(None, target_bir_lowering=False)

    xt = nc.dram_tensor("xt", [KC, P, BC], F32, kind="ExternalInput")
    w1 = nc.dram_tensor("w1", [KC, P, E * H], MM, kind="ExternalInput")
    w2 = nc.dram_tensor("w2", [H, E * O], MM, kind="ExternalInput")
    gw = nc.dram_tensor("gw", [KC, P, E], F32, kind="ExternalInput")
    if has_b1:
        b1d = nc.dram_tensor("b1d", [1, E * H], MM, kind="ExternalInput")
    if has_gb:
        gbd = nc.dram_tensor("gbd", [1, E], F32, kind="ExternalInput")
    if has_b2:
        b2d = nc.dram_tensor("b2d", [E, O], MM, kind="ExternalInput")

    out = nc.dram_tensor("out", [BC, O], F32, kind="ExternalOutput")
    stats = nc.dram_tensor("stats", [P, 17], F32, kind="ExternalOutput")

    with tile.TileContext(nc) as tc:
        with (
            tc.tile_pool(name="wpool", bufs=1) as wpool,
            tc.tile_pool(name="xpool", bufs=3) as xpool,
            tc.tile_pool(name="hpool", bufs=2) as hpool,
            tc.tile_pool(name="spool", bufs=2) as spool,
            tc.tile_pool(name="upool", bufs=3) as upool,
            tc.tile_pool(name="opool", bufs=2) as opool,
            tc.tile_pool(name="ph", bufs=1, space=bass.MemorySpace.PSUM) as php,
            tc.tile_pool(name="pout", bufs=1, space=bass.MemorySpace.PSUM) as poutp,
            tc.tile_pool(name="psc", bufs=2, space=bass.MemorySpace.PSUM) as pscp,
            tc.tile_pool(name="ptr", bufs=1, space=bass.MemorySpace.PSUM) as ptrp,
        ):
            # ---------------- persistent weights ----------------
            w1_s = wpool.tile([P, KC, E * H], MM)
            for k in range(KC):
                nc.sync.dma_start(w1_s[:, k, :], w1[k])
            w2_s = wpool.tile([P, E * O], MM)
            nc.sync.dma_start(w2_s[:], w2[:])
            gw_s = wpool.tile([P, KC, E], F32)
            for k in range(KC):
                nc.sync.dma_start(gw_s[:, k, :], gw[k])
            ident_f = wpool.tile([P, P], F32)
            make_identity(nc, ident_f[:])
            ident_r = wpool.tile([P, P], MM)
            nc.vector.tensor_copy(ident_r[:], ident_f[:])
            ident = ident_r[:]
            if has_b1:
                ones_f = wpool.tile([1, P], F32)
                nc.vector.memset(ones_f[:], 1.0)
                ones_t = wpool.tile([1, P], MM)
                nc.vector.tensor_copy(ones_t[:], ones_f[:])
                ones_r = ones_t[:]
                b1_s = wpool.tile([1, E * H], MM)
                nc.sync.dma_start(b1_s[:], b1d[:])
            if has_gb:
                if not has_b1:
                    ones_f = wpool.tile([1, P], F32)
                    nc.vector.memset(ones_f[:], 1.0)
                    ones_t = wpool.tile([1, P], MM)
                    nc.vector.tensor_copy(ones_t[:], ones_f[:])
                    ones_r = ones_t[:]
                gb_s = wpool.tile([1, E], F32)
                nc.sync.dma_start(gb_s[:], gbd[:])
            if has_b2:
                b2_s = wpool.tile([E, O], MM)
                nc.sync.dma_start(b2_s[:], b2d[:])

            # aux-loss accumulators
            macc = wpool.tile([P, E], F32)
            nc.vector.memset(macc[:], 0.0)
            eacc = wpool.tile([P, 1], F32)
            nc.vector.memset(eacc[:], 0.0)

            NEG = -1e30

            for rep in range(REPEAT):
              if rep > 0:
                nc.vector.memset(macc[:], 0.0)
                nc.vector.memset(eacc[:], 0.0)
              for t in range(NT):
                tsl = bass.ts(t, P)
                # ---------------- load x^T tile ----------------
                xt_t = xpool.tile([P, KC, P], F32)
                for k in range(KC):
                    nc.sync.dma_start(xt_t[:, k, :], xt[k, :, tsl])

                # ---------------- gate + mm1 ----------------
                ps_s = pscp.tile([P, E], F32)
                ps_h = php.tile([P, E * H], F32)
                xr_t = xpool.tile([P, KC, P], MM, tag="xr")
                nc.vector.tensor_copy(xr_t[:], xt_t[:])
                for k in range(KC):
                    lhs = xr_t[:, k, :]
                    nc.tensor.matmul(ps_s[:], xt_t[:, k, :], gw_s[:, k, :],
                                     start=(k == 0), stop=(k == KC - 1 and not has_gb))
                    for g in range(4):
                        nc.tensor.matmul(ps_h[:, bass.ts(g, 512)], lhs,
                                         w1_s[:, k, bass.ts(g, 512)],
                                         start=(k == 0),
                                         stop=(k == KC - 1 and not has_b1))
                if has_gb:
                    nc.tensor.matmul(ps_s[:], ones_f[:], gb_s[:], start=False, stop=True)
                if has_b1:
                    for g in range(4):
                        nc.tensor.matmul(ps_h[:, bass.ts(g, 512)], ones_r,
                                         b1_s[:, bass.ts(g, 512)], start=False, stop=True)

                # ---------------- routing ----------------
                s_t = spool.tile([P, E], F32)
                nc.vector.tensor_copy(s_t[:], ps_s[:])
                t8 = spool.tile([P, 8], F32)
                nc.vector.max(out=t8[:], in_=s_t[:])
                v0 = t8[:, 0:1]
                v1 = t8[:, 1:2]
                # top-2 softmax weights
                dfe = spool.tile([P, 1], F32)
                nc.vector.tensor_tensor(dfe[:], v1, v0, op=AluOpType.subtract)
                ew = spool.tile([P, 1], F32)
                nc.scalar.activation(ew[:], dfe[:], AF.Exp)
                zs = spool.tile([P, 1], F32)
                nc.vector.tensor_scalar_add(zs[:], ew[:], 1.0)
                rz = spool.tile([P, 1], F32)
                nc.vector.reciprocal(rz[:], zs[:])
                w1g = spool.tile([P, 1], F32)
                nc.vector.tensor_tensor(w1g[:], ew[:], rz[:], op=AluOpType.mult)
                w0g = spool.tile([P, 1], F32)
                nc.vector.tensor_scalar(w0g[:], w1g[:], -1.0, 1.0,
                                        op0=AluOpType.mult, op1=AluOpType.add)
                m0 = spool.tile([P, E], F32)
                m1 = spool.tile([P, E], F32)
                nc.vector.tensor_scalar(m0[:], s_t[:], v0, None, op0=AluOpType.is_equal)
                nc.vector.tensor_scalar(m1[:], s_t[:], v1, None, op0=AluOpType.is_equal)
                wg_t = spool.tile([P, E], F32)
                tt0 = spool.tile([P, E], F32)
                nc.vector.tensor_scalar(tt0[:], m0[:], w0g[:], None, op0=AluOpType.mult)
                nc.vector.tensor_scalar(wg_t[:], m1[:], w1g[:], None, op0=AluOpType.mult)
                nc.vector.tensor_add(wg_t[:], wg_t[:], tt0[:])
                # usage accumulation
                nc.vector.tensor_add(macc[:], macc[:], m0[:])
                nc.vector.tensor_add(macc[:], macc[:], m1[:])
                # entropy of full softmax
                nv0 = spool.tile([P, 1], F32)
                nc.vector.tensor_scalar(nv0[:], v0, -1.0, None, op0=AluOpType.mult)
                e1 = spool.tile([P, E], F32)
                zf = spool.tile([P, 1], F32)
                nc.scalar.activation(e1[:], s_t[:], AF.Exp, bias=nv0[:], accum_out=zf[:])
                smul = spool.tile([P, E], F32)
                nc.vector.tensor_tensor(smul[:], s_t[:], e1[:], op=AluOpType.mult)
                tsum = spool.tile([P, 1], F32)
                nc.vector.reduce_sum(tsum[:], smul[:], axis=mybir.AxisListType.X)
                lz = spool.tile([P, 1], F32)
                nc.scalar.activation(lz[:], zf[:], AF.Ln)
                rzf = spool.tile([P, 1], F32)
                nc.vector.reciprocal(rzf[:], zf[:])
                qq = spool.tile([P, 1], F32)
                nc.vector.tensor_tensor(qq[:], tsum[:], rzf[:], op=AluOpType.mult)
                ent = spool.tile([P, 1], F32)
                nc.vector.tensor_tensor(ent[:], v0, lz[:], op=AluOpType.add)
                nc.vector.tensor_tensor(ent[:], ent[:], qq[:], op=AluOpType.subtract)
                nc.vector.tensor_add(eacc[:], eacc[:], ent[:])

                # ---------------- gelu + LN stats ----------------
                hg = hpool.tile([P, E, H], MM)
                S = spool.tile([P, E], F32)
                SS = spool.tile([P, E], F32)
                sqscr = hpool.tile([P, H], MM, tag="sqscr")
                for e in range(E):
                    nc.scalar.activation(hg[:, e, :], ps_h[:, bass.ts(e, H)], GELU_FUNC,
                                         accum_out=S[:, e:e + 1])
                for e in range(E):
                    nc.scalar.activation(sqscr[:], hg[:, e, :], AF.Square,
                                         accum_out=SS[:, e:e + 1])

                # batched LN math: A = wg * rsqrt(var+eps), Cn = mu * A
                mu = spool.tile([P, E], F32)
                nc.vector.tensor_scalar(mu[:], S[:], 1.0 / H, None, op0=AluOpType.mult)
                varpe = spool.tile([P, E], F32)
                nc.vector.tensor_scalar(varpe[:], SS[:], 1.0 / H, LN_EPS,
                                        op0=AluOpType.mult, op1=AluOpType.add)
                musq = spool.tile([P, E], F32)
                nc.vector.tensor_tensor(musq[:], mu[:], mu[:], op=AluOpType.mult)
                nc.vector.tensor_tensor(varpe[:], varpe[:], musq[:], op=AluOpType.subtract)
                rec = spool.tile([P, E], F32)
                nc.vector.reciprocal(rec[:], varpe[:])
                inv = spool.tile([P, E], F32)
                nc.scalar.activation(inv[:], rec[:], AF.Sqrt)
                A = spool.tile([P, E], F32)
                nc.vector.tensor_tensor(A[:], wg_t[:], inv[:], op=AluOpType.mult)
                Cn = spool.tile([P, E], F32)
                nc.vector.tensor_tensor(Cn[:], mu[:], A[:], op=AluOpType.mult)

                # ---------------- u = A*hg - Cn (broadcast along H) ----------------
                u_all = hpool.tile([P, E, H], MM)
                A3 = A[:, :, None].to_broadcast([P, E, H])
                C3 = Cn[:, :, None].to_broadcast([P, E, H])
                nc.vector.tensor_tensor(u_all[:], hg[:], A3, op=AluOpType.mult)
                nc.vector.tensor_tensor(u_all[:], u_all[:], C3, op=AluOpType.subtract)

                # ---------------- transpose + mm2 ----------------
                ps_o = poutp.tile([P, O], F32)
                for e in range(E):
                    ps_t = ptrp.tile([P, P], MM)
                    nc.tensor.transpose(ps_t[:], u_all[:, e, :], ident)
                    uT = upool.tile([P, P], MM)
                    if e % 2 == 0:
                        nc.vector.tensor_copy(uT[:], ps_t[:])
                    else:
                        nc.scalar.copy(uT[:], ps_t[:])
                    nc.tensor.matmul(ps_o[:], uT[:], w2_s[:, bass.ts(e, O)],
                                     start=(e == 0), stop=(e == E - 1 and not has_b2))
                if has_b2:
                    ps_wt = ptrp.tile([P, P], MM, tag="ps_wt")
                    wgr = spool.tile([P, E], MM, tag="wgr")
                    nc.vector.tensor_copy(wgr[:], wg_t[:])
                    nc.tensor.transpose(ps_wt[:E, :], wgr[:], ident)
                    wgT = upool.tile([E, P], MM, tag="wgT")
                    nc.vector.tensor_copy(wgT[:], ps_wt[:E, :])
                    nc.tensor.matmul(ps_o[:], wgT[:], b2_s[:], start=False, stop=True)

                o_t = opool.tile([P, O], F32)
                nc.vector.tensor_copy(o_t[:], ps_o[:])
                nc.sync.dma_start(out[tsl, :], o_t[:])

            # ---------------- stats out ----------------
            nc.sync.dma_start(stats[:, 0:E], macc[:])
            nc.sync.dma_start(stats[:, E:E + 1], eacc[:])

    nc.compile()
    return nc


_CACHE = {}


def _get_program(has_b1, has_gb, has_b2):
    key = (has_b1, has_gb, has_b2)
    if key not in _CACHE:
        _CACHE[key] = build_program(*key)
    return _CACHE[key]


def kernel(x, gate_W, gate_b, W1, b1, ln_g, ln_b, W2, b2):
    x = np.asarray(x, dtype=np.float32)
    gate_W = np.asarray(gate_W, dtype=np.float32)
    gate_b = np.asarray(gate_b, dtype=np.float32)
    W1 = np.asarray(W1, dtype=np.float32)
    b1 = np.asarray(b1, dtype=np.float32)
    ln_g = np.asarray(ln_g, dtype=np.float32)
    ln_b = np.asarray(ln_b, dtype=np.float32)
    W2 = np.asarray(W2, dtype=np.float32)
    b2 = np.asarray(b2, dtype=np.float32)

    has_b1 = bool(np.any(b1 != 0.0))
    has_gb = bool(np.any(gate_b != 0.0))
    # fold ln_g into W2; ln_b and b2 fold into a rank-1 output correction
    W2g = (ln_g[:, :, None] * W2).astype(np.float32)          # [E, H, O]
    B2 = (b2 + np.einsum("eh,eho->eo", ln_b, W2)).astype(np.float32)  # [E, O]
    has_b2 = bool(np.any(B2 != 0.0))

    nc = _get_program(has_b1, has_gb, has_b2)

    # host-side layout prep (weights shared by all cores)
    xT = np.ascontiguousarray(x.T)                             # [D, B]
    w1h = np.ascontiguousarray(
        W1.transpose(1, 0, 2).reshape(D, E * H).reshape(KC, P, E * H))
    w2h = np.ascontiguousarray(W2g.transpose(1, 0, 2).reshape(H, E * O))
    gwh = np.ascontiguousarray(gate_W.reshape(KC, P, E))

    in_maps = []
    for c in range(NCORES):
        m = {
            "xt": np.ascontiguousarray(
                xT[:, c * BC:(c + 1) * BC].reshape(KC, P, BC)),
            "w1": w1h,
            "w2": w2h,
            "gw": gwh,
        }
        if has_b1:
            m["b1d"] = np.ascontiguousarray(b1.reshape(1, E * H))
        if has_gb:
            m["gbd"] = np.ascontiguousarray(gate_b.reshape(1, E))
        if has_b2:
            m["b2d"] = B2
        in_maps.append(m)

    from concourse.bass_utils import run_bass_kernel_spmd
    res = run_bass_kernel_spmd(nc, in_maps, core_ids=list(range(NCORES)))

    out = np.concatenate([np.asarray(r["out"]) for r in res.results], axis=0)

    # aux loss from per-core stats
    usage_counts = np.zeros(E, dtype=np.float64)
    ent_sum = 0.0
    for r in res.results:
        st = np.asarray(r["stats"])
        usage_counts += st[:, 0:E].sum(axis=0, dtype=np.float64)
        ent_sum += st[:, E].sum(dtype=np.float64)
    expert_usage = (usage_counts / B).astype(np.float32)
    load_balance = np.mean((expert_usage - 1.0 / E) ** 2, dtype=np.float32)
    entropy = np.float32(ent_sum / B)
    aux_loss = np.float32(load_balance - 0.1 * entropy)

    return out, aux_loss
